# revision 12
# baseline (speedup 1.0000x reference)
# Trainium2 Bass kernel for nn_MultiHeadAttention (B=2, L=2048, HID=2048, 16 heads).
#
# Sharding: tensor-parallel over heads — 2 heads per NeuronCore x 8 cores.
# Each core computes q/k/v projections for its 2 heads, causal attention,
# and a partial output projection (its 256 input channels of W_out); the
# host sums the 8 partial [B, L, HID] outputs.
#
# Per-core layout choices:
#   - x is shipped pre-transposed per batch (xT[b] = x[b].T, [HID, L], bf16).
#   - Q^T / K^T are computed directly in [D=128, L] layout (head dim on
#     partitions) so score tiles S^T[k,q] come out of the PE ready to be
#     used as PV-matmul stationary operands — no transposes in the softmax
#     path.
#   - RoPE: the head dim of Wq/Wk is permuted even-first on the host, which
#     turns the reference's interleaved rotation into a half-swap + two
#     elementwise multiply-adds with precomputed [128, L] cos/sin tables.
#   - RMSNorm: folded into a per-position scale. The q-side scale (which
#     also absorbs 1/sqrt(D)) is applied to Q^T before RoPE; the k-side
#     scale is applied as the per-partition `scale` operand of the exp
#     activation on S^T tiles.
#   - Softmax denominator: a ones-column appended to V (V_aug[:,128] = 1)
#     so the PV matmul accumulates the denominator for free.
#   - Causal mask: S^T tiles strictly above the diagonal are skipped
#     entirely; diagonal-straddling tiles get a [128,128] additive mask on
#     the diagonal block and a GpSimd zero-fill of the dead columns of E.
#   - All transposes (V^T -> V, attn -> attn^T) run on the DMA engines via
#     dma_start_transpose — zero compute-engine time.
#   - dtypes: bf16 for the big projections and PV (full PE rate), float32r
#     (TF32-like, full rate at N>=512) for the score matmuls.

import numpy as np
import ml_dtypes

B, L, HID, NH, D = 2, 2048, 2048, 16, 128
NCORES = 8
HPC = NH // NCORES          # heads per core = 2
NCH = HID // 128            # 16 contraction chunks
NL = 512                    # L tile (free dim) for projections / S tiles
NLB = L // NL               # 4 L-blocks
ROPE_BASE = 10000.0
EPS = 1e-5
MASK_VAL = -1e9

_BF16 = ml_dtypes.bfloat16
_cache = {}


def _host_constants():
    if "consts" in _cache:
        return _cache["consts"]
    # RoPE tables in the even-first permuted basis.
    i = np.arange(64, dtype=np.float64)
    inv_freq = ROPE_BASE ** (-2.0 * i / D)                     # [64]
    ang = np.arange(L, dtype=np.float64)[:, None] * inv_freq   # [L, 64]
    cos, sin = np.cos(ang).T, np.sin(ang).T                    # [64, L]
    csa = np.concatenate([cos, cos], axis=0).astype(_BF16)    # [128, L]
    csb = np.concatenate([-sin, sin], axis=0).astype(_BF16)   # [128, L]
    _cache["consts"] = (csa, csb)
    return _cache["consts"]


def _build_nc():
    if "nc" in _cache:
        return _cache["nc"]
    import concourse.bass as bass  # noqa: F401
    from concourse import bacc
    import concourse.tile as tile
    import concourse.mybir as mybir
    from contextlib import ExitStack

    f32 = mybir.dt.float32
    f32r = mybir.dt.float32r
    bf16 = mybir.dt.bfloat16
    EXP = mybir.ActivationFunctionType.Exp
    SQRT = mybir.ActivationFunctionType.Sqrt

    nc = bacc.Bacc("TRN2", target_bir_lowering=False, debug=False,
                   enable_asserts=True)
    xT = nc.dram_tensor("xT", [B, HID, L], bf16, kind="ExternalInput").ap()
    wqkvT = nc.dram_tensor("wqkvT", [HID, 6 * D], bf16, kind="ExternalInput").ap()
    woutT = nc.dram_tensor("woutT", [HPC * D, HID], bf16, kind="ExternalInput").ap()
    csa_d = nc.dram_tensor("csa", [D, L], bf16, kind="ExternalInput").ap()
    csb_d = nc.dram_tensor("csb", [D, L], bf16, kind="ExternalInput").ap()
    out_d = nc.dram_tensor("out", [B, L, HID], f32, kind="ExternalOutput").ap()

    with tile.TileContext(nc) as tc, ExitStack() as ctx:
        cpool = ctx.enter_context(tc.tile_pool(name="consts", bufs=1))
        xpool = ctx.enter_context(tc.tile_pool(name="x", bufs=1))
        qkpool = ctx.enter_context(tc.tile_pool(name="qk", bufs=1))
        qk1pool = ctx.enter_context(tc.tile_pool(name="qk1", bufs=1))
        epool = ctx.enter_context(tc.tile_pool(name="e", bufs=16))
        atpool = ctx.enter_context(tc.tile_pool(name="at", bufs=1))
        anpool = ctx.enter_context(tc.tile_pool(name="an", bufs=4))
        spool = ctx.enter_context(tc.tile_pool(name="stage", bufs=2))
        s1pool = ctx.enter_context(tc.tile_pool(name="stage1", bufs=1))
        opool = ctx.enter_context(tc.tile_pool(name="ostage", bufs=2))
        import os
        pb, pс = os.environ.get("PSUM_CFG", "4211").strip(), None
        _b, _s, _o, _m = (int(c) for c in pb)
        ps_big = ctx.enter_context(tc.tile_pool(name="psA", bufs=_b, space="PSUM"))
        ps_s = (ctx.enter_context(tc.tile_pool(name="psSc", bufs=_s, space="PSUM"))
                if _s else None)
        ps_o = ctx.enter_context(tc.tile_pool(name="psO", bufs=_o, space="PSUM"))
        ps_sm = ctx.enter_context(tc.tile_pool(name="psS", bufs=_m, space="PSUM"))
        if ps_s is None:
            ps_s = ps_big

        # ---- constants ----
        # DMA issue order matters (FIFO per ring): the first QKV matmul needs
        # wt group 0 + x group 0 + csb, so those go first on the sync ring;
        # later-needed loads ride the scalar ring.
        wtg = [cpool.tile([128, 4 * 6 * D], bf16, tag=f"wt{g}", name=f"wt{g}")
               for g in range(4)]
        def load_wt(g, eng):
            eng.dma_start(
                wtg[g][:, :].rearrange("p (c f) -> p c f", f=6 * D),
                wqkvT[g * 512:(g + 1) * 512].rearrange("(c p) f -> p c f", p=128))
        load_wt(0, nc.sync)
        xg0 = [xpool.tile([128, 4 * L], bf16, tag=f"xg{g}", name=f"x0g{g}")
               for g in range(4)]
        nc.sync.dma_start(xg0[0][:, :].rearrange("p (c l) -> p c l", l=L),
                          xT[0, 0:512].rearrange("(c p) l -> p c l", p=128))
        csb = cpool.tile([128, L], bf16, tag="csb")
        nc.sync.dma_start(csb[:, :], csb_d[:, :])
        csa = cpool.tile([128, L], bf16, tag="csa")
        nc.sync.dma_start(csa[:, :], csa_d[:, :])
        for g in range(1, 4):
            nc.sync.dma_start(xg0[g][:, :].rearrange("p (c l) -> p c l", l=L),
                              xT[0, g * 512:(g + 1) * 512].rearrange(
                                  "(c p) l -> p c l", p=128))
        for g in range(1, 4):
            load_wt(g, nc.scalar)
        wo = cpool.tile([128, HPC * HID], bf16, tag="wo")
        nc.scalar.dma_start(wo[:, :].rearrange("p (h f) -> p h f", f=HID),
                            woutT.rearrange("(h p) f -> p h f", p=128))
        ident = cpool.tile([128, 128], bf16, tag="ident")
        from concourse.masks import make_identity
        make_identity(nc, ident[:, :])
        # mask128[k, q] = 0 where q >= k else MASK_VAL (strict upper = masked)
        mask128 = cpool.tile([128, 128], f32, tag="mask128")
        nc.gpsimd.memset(mask128[:, :], 0.0)
        nc.gpsimd.affine_select(
            out=mask128[:, :], in_=mask128[:, :],
            compare_op=mybir.AluOpType.is_ge, fill=MASK_VAL,
            base=0, pattern=[[1, 128]], channel_multiplier=-1)
        ones_c32 = cpool.tile([128, 1], f32, tag="ones_c")
        nc.gpsimd.memset(ones_c32[:, :], 1.0)
        ones_c = ones_c32[:, :].bitcast(f32r)
        eps_q = cpool.tile([1, 1], f32, tag="eps_q")
        nc.gpsimd.memset(eps_q[:, :], float(D) * EPS)
        eps_k = cpool.tile([128, 1], f32, tag="eps_k")
        nc.gpsimd.memset(eps_k[:, :], EPS)

        for b in range(B):
            # x[b].T in 4 chunk-groups so matmuls can start before the whole
            # 8 MB arrives. b=0's groups were loaded with the constants.
            if b == 0:
                xg = xg0
            else:
                xg = [xpool.tile([128, 4 * L], bf16, tag=f"xg{g}",
                                 name=f"x{b}g{g}")
                      for g in range(4)]
                for g in range(4):
                    nc.sync.dma_start(
                        xg[g][:, :].rearrange("p (c l) -> p c l", l=L),
                        xT[b, g * 512:(g + 1) * 512].rearrange(
                            "(c p) l -> p c l", p=128))

            attnT = [atpool.tile([128, L], bf16, tag=f"attnT{h}",
                                 name=f"attnT{b}_{h}")
                     for h in range(HPC)]

            for h in range(HPC):
                qr = qkpool.tile([128, L], bf16, tag="qr")
                kr = qkpool.tile([128, L], bf16, tag="kr")
                vT = qk1pool.tile([128, L], bf16, tag="vT")
                va = qk1pool.tile([128, 16 * (D + 1)], bf16, tag="va")
                ckT = qk1pool.tile([128, 16], f32, tag="ckT")

                # ---- q/k/v projections + RMS + RoPE ----
                # Contraction chunk c is the OUTER loop over 4 concurrent
                # PSUM banks: each W chunk is loaded into the PE once per
                # projection (2048 moving rows per Ldweights instead of 512)
                # and the PE streams 16x4 matmuls back-to-back.
                for t, name in ((0, "q"), (1, "k"), (2, "v")):
                    wcol = (3 * h + t) * D
                    pss = [ps_big.tile([128, NL], f32, tag="big",
                                       name=f"pj{b}{h}{t}_{n}")
                           for n in range(NLB)]
                    for c in range(NCH):
                        for n in range(NLB):
                            nc.tensor.matmul(
                                pss[n][:, :],
                                wtg[c // 4][:, (c % 4) * 6 * D + wcol:
                                            (c % 4) * 6 * D + wcol + D],
                                xg[c // 4][:, (c % 4) * L + n * NL:
                                           (c % 4) * L + (n + 1) * NL],
                                start=(c == 0), stop=(c == NCH - 1))
                    for n in range(NLB):
                        ps = pss[n]
                        if name == "v":
                            nc.scalar.copy(vT[:, n * NL:(n + 1) * NL], ps[:, :])
                            continue
                        # RoPE inputs read the PSUM tile directly: half-swap
                        # via ScalarE (GpSimd cannot read PSUM), csb-multiply
                        # on DVE. sumsq is halfswap-invariant, so compute the
                        # square from sw (SBUF) to spare the PSUM read port.
                        sw = spool.tile([128, NL], f32, tag="sw")
                        nc.scalar.copy(sw[0:64, :], ps[64:128, :])
                        nc.scalar.copy(sw[64:128, :], ps[0:64, :])
                        sq = spool.tile([128, NL], f32r, tag="sq")
                        nc.vector.tensor_mul(sq[:, :], sw[:, :], sw[:, :])
                        m2 = spool.tile([128, NL], f32, tag="m2")
                        nc.vector.tensor_mul(m2[:, :], ps[:, :],
                                             csb[:, n * NL:(n + 1) * NL])
                        if name == "q":
                            # c_q = 1/sqrt(sumsq + D*eps)  (includes 1/sqrt(D))
                            rrow = ps_sm.tile([1, NL], f32, tag="sm")
                            nc.tensor.matmul(rrow[:, :], ones_c, sq[:, :],
                                             start=True, stop=True)
                            srow = s1pool.tile([1, NL], f32, tag="srow")
                            nc.scalar.activation(srow[:, :], rrow[:, :], SQRT,
                                                 bias=eps_q[:, :], scale=1.0)
                            cqrow = s1pool.tile([1, NL], f32, tag="cqrow")
                            nc.vector.reciprocal(cqrow[:, :], srow[:, :])
                            bcs = spool.tile([128, NL], f32, tag="bcs")
                            nc.gpsimd.partition_broadcast(bcs[:, :], cqrow[:, :])
                        else:
                            # c_k = 1/sqrt(sumsq/D + eps), in [128, 4] per chunk
                            ckp = ps_sm.tile([128, 4], f32, tag="sm")
                            for i in range(4):
                                nc.tensor.matmul(
                                    ckp[:, i:i + 1],
                                    sq[:, i * 128:(i + 1) * 128].bitcast(f32),
                                    ones_c32[:, :],
                                    start=True, stop=True, skip_group_check=True)
                            cks = s1pool.tile([128, 4], f32, tag="cks")
                            nc.scalar.activation(cks[:, :], ckp[:, :], SQRT,
                                                 bias=eps_k[:, :], scale=1.0 / D)
                            nc.vector.reciprocal(ckT[:, n * 4:(n + 1) * 4],
                                                 cks[:, :])
                        # RoPE: y = csa*halfswap(x) + csb*x  [+ *c_q for q]
                        m1 = spool.tile([128, NL], f32, tag="m1")
                        nc.vector.tensor_mul(m1[:, :], sw[:, :],
                                             csa[:, n * NL:(n + 1) * NL])
                        dst = qr if name == "q" else kr
                        if name == "q":
                            y = spool.tile([128, NL], f32, tag="y")
                            nc.vector.tensor_add(y[:, :], m1[:, :], m2[:, :])
                            nc.vector.tensor_mul(dst[:, n * NL:(n + 1) * NL],
                                                 y[:, :], bcs[:, :])
                        else:
                            nc.vector.tensor_add(dst[:, n * NL:(n + 1) * NL],
                                                 m1[:, :], m2[:, :])

                # ---- V^T -> V natural (DMA transpose) with ones column ----
                # PE transposes (DMA transposes serialize against copy-mode
                # DMAs via the xbar-mode hazard, stalling everything).
                nc.gpsimd.memset(va[:, :], 1.0)
                for lc in range(16):
                    vtp = ps_sm.tile([128, 128], bf16, tag="sm",
                                     name=f"vtp{b}{h}_{lc}")
                    nc.tensor.transpose(vtp[:, :], vT[:, lc * 128:(lc + 1) * 128],
                                        ident[:, :])
                    nc.vector.tensor_copy(va[:, lc * 129: lc * 129 + 128],
                                          vtp[:, :])

                # ---- attention ----
                atns = []
                for J in range(NLB):
                    etiles = []
                    for c in range(4 * J + 4):
                        r = c - 4 * J
                        et = epool.tile([128, NL], bf16, tag="e",
                                        name=f"e{b}{h}{J}_{c}")
                        if r >= 0:
                            # diagonal-straddling tile: columns below
                            # q = 128r are fully masked — skip them in the
                            # matmul; mask the diagonal 128-block; zero-fill
                            # the dead prefix of E.
                            w = NL - r * 128
                            sp = ps_s.tile([128, NL], f32,
                                           tag="s" if ps_s is not ps_big else "big",
                                           name=f"spd{b}{h}{J}_{c}")
                            nc.tensor.matmul(
                                sp[:, 0:w], kr[:, c * 128:(c + 1) * 128],
                                qr[:, J * NL + r * 128:(J + 1) * NL],
                                start=True, stop=True)
                            nc.vector.tensor_add(sp[:, 0:128], sp[:, 0:128],
                                                 mask128[:, :])
                            if r > 0:
                                nc.gpsimd.memset(et[:, 0:r * 128], 0.0)
                            nc.scalar.activation(et[:, r * 128:], sp[:, 0:w],
                                                 EXP, scale=ckT[:, c:c + 1])
                        else:
                            sp = ps_s.tile([128, NL], f32,
                                           tag="s" if ps_s is not ps_big else "big",
                                           name=f"sp{b}{h}{J}_{c}")
                            nc.tensor.matmul(sp[:, :], kr[:, c * 128:(c + 1) * 128],
                                             qr[:, J * NL:(J + 1) * NL],
                                             start=True, stop=True)
                            nc.scalar.activation(et[:, :], sp[:, :],
                                                 EXP, scale=ckT[:, c:c + 1])
                        etiles.append(et)
                    for si in range(4):
                        s = 4 * J + si
                        op = ps_o.tile([128, D + 1], f32, tag="o")
                        for c in range(s + 1):
                            nc.tensor.matmul(
                                op[:, :],
                                etiles[c][:, si * 128:(si + 1) * 128],
                                va[:, c * 129:(c + 1) * 129],
                                start=(c == 0), stop=(c == s))
                        rden = s1pool.tile([128, 1], f32, tag="rden")
                        nc.vector.reciprocal(rden[:, :], op[:, D:D + 1])
                        atn = anpool.tile([128, 128], bf16, tag="atn",
                                          name=f"atn{b}{h}_{s}")
                        nc.vector.tensor_scalar_mul(atn[:, :], op[:, 0:D],
                                                    rden[:, :])
                        atp = ps_sm.tile([128, 128], bf16, tag="sm",
                                         name=f"atp{b}{h}_{s}")
                        nc.tensor.transpose(atp[:, :], atn[:, :], ident[:, :])
                        nc.vector.tensor_copy(
                            attnT[h][:, s * 128:(s + 1) * 128], atp[:, :])

            # ---- output projection (partial over this core's channels) ----
            # Stores go out as 2 MB SWDGE (gpsimd) DMAs with 8 KB/partition
            # descriptor lines: HWDGE stores measure ~40 GB/s flat while
            # SWDGE stores with >=8 KB lines are ~5-10x that.
            for sg in range(L // 256):
                ob = opool.tile([128, 2 * HID], f32, tag="ob")
                for cq in range(2):
                    qb = sg * 2 + cq
                    for f in range(NLB):
                        opj = ps_big.tile([128, NL], f32, tag="big")
                        for hh in range(HPC):
                            nc.tensor.matmul(
                                opj[:, :],
                                attnT[hh][:, qb * 128:(qb + 1) * 128],
                                wo[:, hh * HID + f * NL:
                                   hh * HID + (f + 1) * NL],
                                start=(hh == 0), stop=(hh == HPC - 1))
                        # alternate PSUM-drain engines so neither DVE nor
                        # Act serializes the store tail
                        ceng = nc.vector if f % 2 == 0 else nc.scalar
                        if ceng is nc.vector:
                            ceng.tensor_copy(
                                ob[:, (cq * 4 + f) * NL:
                                   (cq * 4 + f + 1) * NL],
                                opj[:, :])
                        else:
                            ceng.copy(
                                ob[:, (cq * 4 + f) * NL:
                                   (cq * 4 + f + 1) * NL],
                                opj[:, :])
                nc.gpsimd.dma_start(
                    out_d[b, sg * 256:(sg + 1) * 256, :].rearrange(
                        "(c p) f -> p c f", p=128),
                    ob[:, :].rearrange("p (c f) -> p c f", f=HID))

    nc.compile()
    _cache["nc"] = nc
    return nc


def _prep_in_maps(x, W_qkv, W_out):
    csa, csb = _host_constants()
    xT = np.ascontiguousarray(x.transpose(0, 2, 1)).astype(_BF16)
    perm = np.concatenate([np.arange(0, D, 2), np.arange(1, D, 2)])
    in_maps = []
    for core in range(NCORES):
        h0 = HPC * core
        blocks = []
        for h in (h0, h0 + 1):
            wq = W_qkv[h * D:(h + 1) * D, :][perm]
            wk = W_qkv[HID + h * D: HID + (h + 1) * D, :][perm]
            wv = W_qkv[2 * HID + h * D: 2 * HID + (h + 1) * D, :]
            blocks += [wq, wk, wv]
        wqkvT = np.ascontiguousarray(
            np.concatenate(blocks, axis=0).T).astype(_BF16)
        woutT = np.ascontiguousarray(
            W_out[:, h0 * D:(h0 + HPC) * D].T).astype(_BF16)
        in_maps.append({
            "xT": xT, "wqkvT": wqkvT, "woutT": woutT,
            "csa": csa, "csb": csb,
        })
    return in_maps


def kernel(x, W_qkv, W_out):
    from concourse.bass_utils import run_bass_kernel_spmd
    nc = _build_nc()
    in_maps = _prep_in_maps(np.asarray(x, dtype=np.float32),
                            np.asarray(W_qkv, dtype=np.float32),
                            np.asarray(W_out, dtype=np.float32))
    res = run_bass_kernel_spmd(nc, in_maps, core_ids=list(range(NCORES)))
    out = res.results[0]["out"].astype(np.float64)
    for r in res.results[1:]:
        out += r["out"]
    return out.astype(np.float32)

